# revision 10
# baseline (speedup 1.0000x reference)
"""8x8 block 2D DCT (orthonormal DCT-II) on x[32,3,512,512] f32, 8 NeuronCores.

Data parallel: batch dim 32 -> 4 per core. Each core's shard is viewed as
[6144, 512] rows; every 128 consecutive rows (16 H-blocks) form a chunk.

Per [128,128] tile T (partitions = 128 H-rows), with R = kron(I16, D).T in
SBUF, two transposing matmuls compute the 2D DCT without any explicit
transpose:
  mm1: P1 = T.T @ R = (BD @ T).T      (H-pass, lands transposed)
  mm2: P2 = P1.T @ R = BD @ T @ BD.T  (W-pass, correct orientation)
where BD = kron(I16, D). The stationary operand is the data/intermediate
(lhsT), exploiting that the PE transposes the stationary operand for free.

Matmuls run in float32r (single-pass fp32, ~2^-13 precision, 1.9x faster
than the 2-pass float32 mode). The PSUM->SBUF copies are split: ACT makes
the f32r-rounded mm2 stationary, DVE makes the fp32 output tile.
"""

import numpy as np

BLOCK = 8
B, C, H, W = 32, 3, 512, 512
N_CORES = 8
ROWS_PER_CORE = (B // N_CORES) * C * H  # 6144
N_CHUNKS = ROWS_PER_CORE // 128  # 48
DMA_BATCH = 2  # chunks per dma_start
LOOKAHEAD = 4  # stage-2 lags stage-1 by this many chunks (PE pipelining)

_CACHE = {}


def _dct_matrix(n: int) -> np.ndarray:
    k = np.arange(n)[:, None].astype(np.float64)
    m = np.arange(n)[None, :].astype(np.float64)
    D = np.cos(np.pi * (2.0 * m + 1.0) * k / (2.0 * n))
    D *= np.sqrt(2.0 / n)
    D[0, :] *= np.sqrt(0.5)
    return D.astype(np.float32)


def _r_const() -> np.ndarray:
    BD = np.kron(np.eye(128 // BLOCK, dtype=np.float32), _dct_matrix(BLOCK))
    rt = BD.T
    return np.ascontiguousarray(np.concatenate([rt, rt], axis=1))


def _build_nc():
    import concourse.bacc as bacc
    import concourse.mybir as mybir
    import concourse.tile as tile
    from concourse.bass import ts

    F32 = mybir.dt.float32
    F32R = mybir.dt.float32r

    nc = bacc.Bacc("TRN2", target_bir_lowering=False, debug=False)
    x = nc.dram_tensor("x", [N_CHUNKS, 128, W], F32R, kind="ExternalInput")
    r = nc.dram_tensor("r", [128, 256], F32R, kind="ExternalInput")
    out = nc.dram_tensor("out", [N_CHUNKS, 128, W], F32, kind="ExternalOutput")

    nb = N_CHUNKS // DMA_BATCH
    with tile.TileContext(nc) as tc:
        with (
            tc.tile_pool(name="const", bufs=1) as cpool,
            tc.tile_pool(name="xin", bufs=4) as xpool,
            tc.tile_pool(name="ys", bufs=LOOKAHEAD + 3) as ypool,
            tc.tile_pool(name="zout", bufs=3) as zpool,
            tc.tile_pool(name="ps1", bufs=2, space="PSUM") as ppool1,
            tc.tile_pool(name="ps2", bufs=2, space="PSUM") as ppool2,
        ):
            r_sb = cpool.tile([128, 256], F32R)
            nc.sync.dma_start(out=r_sb, in_=r[:, :])

            zt_tiles = {}
            stage2_q = []

            def stage1(c, xt):
                cc = c % DMA_BATCH
                p1 = ppool1.tile([128, 4, 256], F32, tag="p1")
                for t in range(4):
                    nc.tensor.matmul(p1[:, t, :],
                                     lhsT=xt[:, cc, ts(t, 128)],
                                     rhs=r_sb, start=(t % 2 == 0),
                                     stop=(t % 2 == 1))
                ys = ypool.tile([128, 4, 128], F32R, tag="ys")
                nc.scalar.copy(ys, p1[:, :, 0:128])
                stage2_q.append((c, ys))

            def stage2():
                c, ys = stage2_q.pop(0)
                g, cc = divmod(c, DMA_BATCH)
                p2 = ppool2.tile([128, 4, 256], F32, tag="p2")
                for t in range(4):
                    nc.tensor.matmul(p2[:, t, :],
                                     lhsT=ys[:, t, :],
                                     rhs=r_sb, start=(t % 2 == 0),
                                     stop=(t % 2 == 1))
                nc.vector.tensor_copy(
                    zt_tiles[g][:, cc, :].rearrange("p (a b) -> p a b", b=128),
                    p2[:, :, 0:128])
                if cc == DMA_BATCH - 1:
                    nc.sync.dma_start(
                        out=out[g * DMA_BATCH:(g + 1) * DMA_BATCH].rearrange(
                            "c p w -> p c w"),
                        in_=zt_tiles.pop(g))

            for g in range(nb):
                xt = xpool.tile([128, DMA_BATCH, W], F32R, tag="xt")
                nc.sync.dma_start(
                    out=xt,
                    in_=x[g * DMA_BATCH:(g + 1) * DMA_BATCH].rearrange(
                        "c p w -> p c w"))
                zt_tiles[g] = zpool.tile([128, DMA_BATCH, W], F32, tag="zt", name="zt")
                for cc in range(DMA_BATCH):
                    stage1(g * DMA_BATCH + cc, xt)
                    while len(stage2_q) > LOOKAHEAD:
                        stage2()
            while stage2_q:
                stage2()
    nc.compile()
    return nc


def _get_nc():
    if "nc" not in _CACHE:
        _CACHE["nc"] = _build_nc()
    return _CACHE["nc"]


def kernel(x: np.ndarray, _trace: bool = False, _results_box: list | None = None
           ) -> np.ndarray:
    from concourse import bass_utils

    x = np.asarray(x, dtype=np.float32)
    assert x.shape == (B, C, H, W), x.shape
    nc = _get_nc()
    r_np = _r_const()
    per = B // N_CORES
    shards = [
        np.ascontiguousarray(
            x[i * per:(i + 1) * per].reshape(N_CHUNKS, 128, W))
        for i in range(N_CORES)
    ]
    in_maps = [{"x": s, "r": r_np} for s in shards]
    res = bass_utils.run_bass_kernel_spmd(
        nc, in_maps, core_ids=list(range(N_CORES)), trace=_trace)
    if _results_box is not None:
        _results_box.append(res)
    out = np.empty((B, C, H, W), dtype=np.float32)
    for i in range(N_CORES):
        out[i * per:(i + 1) * per] = res.results[i]["out"].reshape(per, C, H, W)
    return out


# revision 11
# speedup vs baseline: 1.1415x; 1.1415x over previous
"""8x8 block 2D DCT (orthonormal DCT-II) on x[32,3,512,512] f32, 8 NeuronCores.

Data parallel: batch dim 32 -> 4 per core. Each core's shard is viewed as
[6144, 512] rows; every 128 consecutive rows (16 H-blocks) form a chunk.

Per [128,128] tile T (partitions = 128 H-rows), with R = kron(I16, D).T in
SBUF, two transposing matmuls compute the 2D DCT without any explicit
transpose:
  mm1: P1 = T.T @ R = (BD @ T).T      (H-pass, lands transposed)
  mm2: P2 = P1.T @ R = BD @ T @ BD.T  (W-pass, correct orientation)
where BD = kron(I16, D). The stationary operand is the data/intermediate
(lhsT), exploiting that the PE transposes the stationary operand for free.

Matmuls run in float32r (single-pass fp32, ~2^-13 precision, 1.9x faster
than the 2-pass float32 mode). The PSUM->SBUF copies are split: ACT makes
the f32r-rounded mm2 stationary, DVE makes the fp32 output tile.
"""

import numpy as np

BLOCK = 8
B, C, H, W = 32, 3, 512, 512
N_CORES = 8
ROWS_PER_CORE = (B // N_CORES) * C * H  # 6144
N_CHUNKS = ROWS_PER_CORE // 128  # 48
DMA_BATCH = 2  # chunks per dma_start
LOOKAHEAD = 4  # stage-2 lags stage-1 by this many chunks (PE pipelining)

_CACHE = {}


def _dct_matrix(n: int) -> np.ndarray:
    k = np.arange(n)[:, None].astype(np.float64)
    m = np.arange(n)[None, :].astype(np.float64)
    D = np.cos(np.pi * (2.0 * m + 1.0) * k / (2.0 * n))
    D *= np.sqrt(2.0 / n)
    D[0, :] *= np.sqrt(0.5)
    return D.astype(np.float32)


def _r_const() -> np.ndarray:
    BD = np.kron(np.eye(128 // BLOCK, dtype=np.float32), _dct_matrix(BLOCK))
    rt = BD.T
    return np.ascontiguousarray(np.concatenate([rt, rt], axis=1))


def _build_nc():
    import concourse.bacc as bacc
    import concourse.mybir as mybir
    import concourse.tile as tile
    from concourse.bass import ts

    F32 = mybir.dt.float32
    F32R = mybir.dt.float32r

    nc = bacc.Bacc("TRN2", target_bir_lowering=False, debug=False)
    x = nc.dram_tensor("x", [N_CHUNKS, 128, W], F32R, kind="ExternalInput")
    r = nc.dram_tensor("r", [128, 256], F32R, kind="ExternalInput")
    out = nc.dram_tensor("out", [N_CHUNKS, 128, W], F32, kind="ExternalOutput")

    nb = N_CHUNKS // DMA_BATCH
    with tile.TileContext(nc) as tc:
        with (
            tc.tile_pool(name="const", bufs=1) as cpool,
            tc.tile_pool(name="xin", bufs=4) as xpool,
            tc.tile_pool(name="ys", bufs=LOOKAHEAD + 3) as ypool,
            tc.tile_pool(name="zout", bufs=3) as zpool,
            tc.tile_pool(name="ps1", bufs=2, space="PSUM") as ppool1,
            tc.tile_pool(name="ps2", bufs=2, space="PSUM") as ppool2,
        ):
            r_sb = cpool.tile([128, 256], F32R)
            nc.sync.dma_start(out=r_sb, in_=r[:, :])

            zt_tiles = {}
            stage2_q = []

            def stage1(c, xt):
                cc = c % DMA_BATCH
                p1 = ppool1.tile([128, 4, 256], F32, tag="p1")
                for t in range(4):
                    nc.tensor.matmul(p1[:, t, :],
                                     lhsT=xt[:, cc, ts(t, 128)],
                                     rhs=r_sb, start=(t % 2 == 0),
                                     stop=(t % 2 == 1))
                ys = ypool.tile([128, 4, 128], F32R, tag="ys")
                nc.scalar.copy(ys, p1[:, :, 0:128])
                stage2_q.append((c, ys))

            def stage2():
                c, ys = stage2_q.pop(0)
                g, cc = divmod(c, DMA_BATCH)
                p2 = ppool2.tile([128, 4, 256], F32, tag="p2")
                for t in range(4):
                    nc.tensor.matmul(p2[:, t, :],
                                     lhsT=ys[:, t, :],
                                     rhs=r_sb, start=(t % 2 == 0),
                                     stop=(t % 2 == 1))
                nc.vector.tensor_copy(
                    zt_tiles[g][:, cc, :].rearrange("p (a b) -> p a b", b=128),
                    p2[:, :, 0:128])
                if cc == DMA_BATCH - 1:
                    nc.scalar.dma_start(
                        out=out[g * DMA_BATCH:(g + 1) * DMA_BATCH].rearrange(
                            "c p w -> p c w"),
                        in_=zt_tiles.pop(g))

            for g in range(nb):
                xt = xpool.tile([128, DMA_BATCH, W], F32R, tag="xt")
                nc.sync.dma_start(
                    out=xt,
                    in_=x[g * DMA_BATCH:(g + 1) * DMA_BATCH].rearrange(
                        "c p w -> p c w"))
                zt_tiles[g] = zpool.tile([128, DMA_BATCH, W], F32, tag="zt", name="zt")
                for cc in range(DMA_BATCH):
                    stage1(g * DMA_BATCH + cc, xt)
                    while len(stage2_q) > LOOKAHEAD:
                        stage2()
            while stage2_q:
                stage2()
    nc.compile()
    return nc


def _get_nc():
    if "nc" not in _CACHE:
        _CACHE["nc"] = _build_nc()
    return _CACHE["nc"]


def kernel(x: np.ndarray, _trace: bool = False, _results_box: list | None = None
           ) -> np.ndarray:
    from concourse import bass_utils

    x = np.asarray(x, dtype=np.float32)
    assert x.shape == (B, C, H, W), x.shape
    nc = _get_nc()
    r_np = _r_const()
    per = B // N_CORES
    shards = [
        np.ascontiguousarray(
            x[i * per:(i + 1) * per].reshape(N_CHUNKS, 128, W))
        for i in range(N_CORES)
    ]
    in_maps = [{"x": s, "r": r_np} for s in shards]
    res = bass_utils.run_bass_kernel_spmd(
        nc, in_maps, core_ids=list(range(N_CORES)), trace=_trace)
    if _results_box is not None:
        _results_box.append(res)
    out = np.empty((B, C, H, W), dtype=np.float32)
    for i in range(N_CORES):
        out[i * per:(i + 1) * per] = res.results[i]["out"].reshape(per, C, H, W)
    return out
